# revision 1
# baseline (speedup 1.0000x reference)
"""DirectedGraphLayer (GNN message passing) on 8 Trainium2 NeuronCores.

out = relu(x @ W_self + b_self + segment_sum(edge_val * (x@W)[edge_col], edge_row))

v2 changes over baseline:
  - x_trans pipeline in bf16: stage/AllGather/gathers/selector/matmuls are
    bf16 (PSUM accumulation stays fp32).  Halves AllGather + gather traffic,
    4x faster PE matmuls, 2x faster DVE selector builds.
  - 4 SWDGE queues; edge gathers round-robin across queue_num 0..3 so
    descriptor generation and DMA rings run concurrently.

Strategy (node/row parallelism):
  - Destination nodes are partitioned across 8 cores (6250 each, padded to
    6400 = 50 tiles x 128).  Each core computes x_trans = x @ [W | W_self] for
    its own shard (host pre-transposes x; stationary-weight matmuls), then an
    AllGather replicates bf16 x_trans (node-major rows of B*FOUT = 128 bf16 =
    256B).
  - Edges are sharded by destination and split per destination-tile into three
    source classes: LOCAL (source in this core's shard -> gathered from the
    core-local ag_in buffer, overlapping the AllGather), LO / HI remote
    (gathered from the AllGather output; two base offsets because dma_gather
    indices are int16).
  - Per 128-edge chunk a one-hot selector S (S[e,d] = val_e if dest(e)==d,
    built in one DVE op from iota==slot) feeds the tensor engine:
    psum += S.T @ gathered_rows.  Local partial sums drain to SBUF on top of
    the self term + bias; remote chunks accumulate in PSUM; the combine is
    relu(psum + self_buf) written per tile.
"""

import os

import numpy as np

import concourse.bacc as bacc
import concourse.mybir as mybir
import concourse.tile as tile
from concourse.bass_utils import run_bass_kernel_spmd

NCORES = 8
FIN = 128
FOUT = 64
B = 2
GROUP = 2      # dest tiles per remote gather pair
LOCAL_SPLIT = False  # all sources via ag_out; balanced classes, less padding
N_LOC_GATHERS = 5
XT_SLICES = 8
AGIN_SLICES = 4
NQ = 4         # SWDGE queues for edge gathers

V_SKIP_AG = os.environ.get("V_SKIP_AG", "") == "1"
V_SKIP_GATHERS = os.environ.get("V_SKIP_GATHERS", "") == "1"
V_SKIP_COMPUTE = os.environ.get("V_SKIP_COMPUTE", "") == "1"

BF16 = mybir.dt.bfloat16
NP_BF16 = mybir.dt.np(BF16)


def _plan(N, edge_row, edge_col, edge_val):
    """Host-side edge partitioning. Classes: 0=lo-remote, 1=hi-remote, 2=local."""
    npc = -(-N // NCORES)
    tiles = -(-npc // 128)
    if tiles % GROUP:
        tiles += GROUP - tiles % GROUP
    npc_pad = tiles * 128
    rows_all = NCORES * npc_pad
    split = rows_all // 2
    assert split <= 32767 and rows_all - split <= 32768 and npc_pad <= 32767

    dcore = edge_row // npc
    d_local = edge_row - dcore * npc
    t_of_e = d_local // 128
    slot = d_local % 128
    scoreq = ((edge_col // npc) == dcore) & LOCAL_SPLIT
    src_glob = (edge_col // npc) * npc_pad + (edge_col % npc)
    cls = np.where(scoreq, 2, (src_glob >= split).astype(np.int64))

    key = (dcore * tiles + t_of_e) * 3 + cls
    order = np.argsort(key, kind="stable")
    key_s = key[order]
    nkeys = NCORES * tiles * 3
    counts = np.bincount(key_s, minlength=nkeys).reshape(NCORES, tiles, 3)

    pad = counts.max(axis=0)                       # (tiles, 3)
    pad = ((pad + 127) // 128) * 128
    pad[:, 0:2] = np.maximum(pad[:, 0:2], 128)     # lo/hi hold start/stop flags
    pad_lo, pad_hi, pad_loc = pad[:, 0], pad[:, 1], pad[:, 2]

    loc_edges = int(pad_loc.sum())
    rem_edges = int((pad_lo + pad_hi).sum())
    per_core_edges = loc_edges + rem_edges
    nchunks = per_core_edges // 128
    lc_chunks = loc_edges // 128

    # layout: [local blocks tile-major][per group: t0lo,t1lo,t0hi,t1hi]
    block_off = np.zeros((tiles, 3), dtype=np.int64)
    run = 0
    chunk_tile = []
    loc_first = []
    loc_last = []
    loc_gathers = []    # (n_idx, chunk_base)
    for t in range(tiles):
        block_off[t, 2] = run
        cnt = int(pad_loc[t])
        for c in range(cnt // 128):
            chunk_tile.append(t)
            loc_first.append(c == 0)
            loc_last.append(c == cnt // 128 - 1)
        run += cnt
    # split local chunks into N_LOC_GATHERS contiguous gathers on tile bounds
    if loc_edges:
        tgt = loc_edges / N_LOC_GATHERS
        g_start = 0
        acc = 0
        for t in range(tiles):
            acc += int(pad_loc[t])
            if (acc - g_start >= tgt and len(loc_gathers) < N_LOC_GATHERS - 1) \
                    or t == tiles - 1:
                loc_gathers.append((acc - g_start, g_start // 128))
                g_start = acc
        assert sum(n for n, _ in loc_gathers) == loc_edges

    rem_first = []
    rem_last = []
    rem_gathers = []    # (half, n_idx, chunk_base)
    for g in range(tiles // GROUP):
        ts = range(g * GROUP, (g + 1) * GROUP)
        for h in (0, 1):
            n = int(sum((pad_lo if h == 0 else pad_hi)[t] for t in ts))
            rem_gathers.append((h, n, run // 128))
            for t in ts:
                block_off[t, h] = run
                cnt = int((pad_lo if h == 0 else pad_hi)[t])
                for c in range(cnt // 128):
                    chunk_tile.append(t)
                    rem_first.append(h == 0 and c == 0)
                    rem_last.append(h == 1 and c == cnt // 128 - 1)
                run += cnt
    assert run == per_core_edges and len(chunk_tile) == nchunks

    # scatter real edges into the padded per-core layout
    first_of_key = np.zeros(nkeys + 1, dtype=np.int64)
    np.cumsum(np.bincount(key_s, minlength=nkeys), out=first_of_key[1:])
    rank = np.arange(len(key_s)) - first_of_key[key_s]
    c_s = dcore[order]
    t_s = t_of_e[order]
    cl_s = cls[order]
    pos = block_off[t_s, cl_s] + rank

    idx_vals = np.zeros((NCORES, per_core_edges), dtype=np.int16)
    slot_arr = np.zeros((NCORES, per_core_edges), dtype=np.float32)
    val_arr = np.zeros((NCORES, per_core_edges), dtype=np.float32)
    src_s = src_glob[order]
    loc_row = (edge_col % npc)[order]
    src_rel = np.where(cl_s == 2, loc_row,
                       np.where(cl_s == 1, src_s - split, src_s)).astype(np.int16)
    idx_vals[c_s, pos] = src_rel
    slot_arr[c_s, pos] = slot[order].astype(np.float32)
    val_arr[c_s, pos] = edge_val[order]

    slot_t = slot_arr.reshape(NCORES, nchunks, 128).transpose(0, 2, 1).copy()
    val_t = val_arr.reshape(NCORES, nchunks, 128).transpose(0, 2, 1).copy()

    s_total = per_core_edges // 16
    idx_t = np.zeros((NCORES, 128, s_total), dtype=np.int16)
    all_gathers = list(loc_gathers) + [(n, cb) for (_h, n, cb) in rem_gathers]
    for (n, cb) in all_gathers:
        if n == 0:
            continue
        blk = idx_vals[:, cb * 128: cb * 128 + n]
        wrapped = blk.reshape(NCORES, n // 16, 16).transpose(0, 2, 1)
        idx_t[:, :, cb * 8: cb * 8 + n // 16] = np.tile(wrapped, (1, 8, 1))

    return dict(
        npc=npc, tiles=tiles, npc_pad=npc_pad, rows_all=rows_all, split=split,
        nchunks=nchunks, lc_chunks=lc_chunks, s_total=s_total,
        loc_gathers=loc_gathers, rem_gathers=rem_gathers,
        chunk_tile=chunk_tile, loc_first=loc_first, loc_last=loc_last,
        rem_first=rem_first, rem_last=rem_last,
        idx_t=idx_t, slot_t=slot_t, val_t=val_t,
    )


def _build(plan, single_core=False, repeat=1):
    tiles, npc_pad = plan["tiles"], plan["npc_pad"]
    rows_all, split = plan["rows_all"], plan["split"]
    nchunks, lc_chunks, s_total = plan["nchunks"], plan["lc_chunks"], plan["s_total"]
    f32 = mybir.dt.float32

    nc = bacc.Bacc("TRN2", target_bir_lowering=False,
                   num_devices=1 if single_core else NCORES,
                   num_swdge_queues=NQ)
    xT_in = nc.dram_tensor("xT", [128, B * npc_pad], BF16, kind="ExternalInput")
    wcat_in = nc.dram_tensor("wcat", [128, 128], BF16, kind="ExternalInput")
    bias_in = nc.dram_tensor("bias", [128, FOUT], f32, kind="ExternalInput")
    iota_in = nc.dram_tensor("iota", [128, 128], BF16, kind="ExternalInput")
    idx_in = nc.dram_tensor("idx", [128, s_total], mybir.dt.int16, kind="ExternalInput")
    slot_in = nc.dram_tensor("slot", [128, nchunks], f32, kind="ExternalInput")
    val_in = nc.dram_tensor("val", [128, nchunks], f32, kind="ExternalInput")
    if single_core:
        xtf_in = nc.dram_tensor("xtf", [rows_all, 128], BF16, kind="ExternalInput")
    out_d = nc.dram_tensor("out", [npc_pad, 128], f32, kind="ExternalOutput")

    qn = [0]

    def next_q():
        q = qn[0]
        qn[0] = (qn[0] + 1) % NQ
        return q

    with tile.TileContext(nc) as tc:
        with (
            tc.tile_pool(name="persist", bufs=1) as pp,
            tc.tile_pool(name="dram", bufs=1, space="DRAM") as dram,
        ):
            wcat = pp.tile([128, 128], BF16)
            nc.sync.dma_start(wcat[:], wcat_in.ap())
            bias = pp.tile([128, FOUT], f32)
            nc.sync.dma_start(bias[:], bias_in.ap())
            iota = pp.tile([128, 128], BF16)
            nc.sync.dma_start(iota[:], iota_in.ap())
            idx = pp.tile([128, s_total], mybir.dt.int16)
            nc.sync.dma_start(idx[:], idx_in.ap())
            slot = pp.tile([128, nchunks], f32)
            nc.sync.dma_start(slot[:], slot_in.ap())
            val = pp.tile([128, nchunks], f32)
            nc.sync.dma_start(val[:], val_in.ap())
            self_buf = pp.tile([128, npc_pad], f32)   # self + bias + local agg

            def one_pass():
                ag_in = dram.tile([npc_pad, 128], BF16)
                if not single_core:
                    ag_out = dram.tile([rows_all, 128], BF16, addr_space="Shared")
                # ---- phase 1: own-shard x_trans/self, streamed ----
                with (
                    tc.tile_pool(name="ph1", bufs=3) as p1,
                    tc.tile_pool(name="mmps", bufs=6, space="PSUM") as mmps,
                ):
                    stage = p1.tile([128, tiles, 128], BF16, tag="stage", bufs=1)
                    t_per_s = -(-tiles // XT_SLICES)
                    agin_done = 0
                    done_t = 0
                    for s in range(XT_SLICES):
                        t0, t1 = s * t_per_s, min(tiles, (s + 1) * t_per_s)
                        if t0 >= t1:
                            continue
                        w = (t1 - t0) * B * 128
                        xts = p1.tile([128, w], BF16, tag="xts")
                        nc.sync.dma_start(xts[:], xT_in[:, t0 * B * 128: t0 * B * 128 + w])
                        for t in range(t0, t1):
                            for b in range(B):
                                mm = mmps.tile([128, 128], f32, tag="mm")
                                o = ((t - t0) * B + b) * 128
                                nc.tensor.matmul(mm[:], xts[:, o:o + 128], wcat[:],
                                                 start=True, stop=True)
                                nc.scalar.copy(
                                    stage[:, t, b * FOUT:(b + 1) * FOUT], mm[:, 0:FOUT])
                                nc.vector.tensor_add(
                                    self_buf[:, t * 128 + b * FOUT: t * 128 + (b + 1) * FOUT],
                                    mm[:, FOUT:128], bias[:])
                        done_t = t1
                        tgt_t = done_t * AGIN_SLICES // tiles * tiles // AGIN_SLICES
                        if tgt_t > agin_done:
                            nc.sync.dma_start(
                                ag_in[agin_done * 128: tgt_t * 128, :]
                                .rearrange("(t p) c -> p t c", p=128),
                                stage[:, agin_done:tgt_t, :])
                            agin_done = tgt_t
                    if agin_done < tiles:
                        nc.sync.dma_start(
                            ag_in[agin_done * 128:, :].rearrange("(t p) c -> p t c", p=128),
                            stage[:, agin_done:, :])

                # ---- phase 2a: local gathers, issued before the AllGather so
                # the Pool queue is not blocked behind the collective ----
                with (
                    tc.tile_pool(name="locb", bufs=1) as locbp,
                    tc.tile_pool(name="sel", bufs=10) as selp,
                    tc.tile_pool(name="locps", bufs=3, space="PSUM") as locps,
                    tc.tile_pool(name="accps", bufs=4, space="PSUM") as accps,
                    tc.tile_pool(name="ph2", bufs=3) as p2,
                    tc.tile_pool(name="outp", bufs=4) as outp,
                ):
                    if lc_chunks:
                        locb = locbp.tile([128, lc_chunks, 128], BF16)
                        if V_SKIP_GATHERS:
                            nc.vector.memset(locb[:], 0.0)
                        for (n, cb) in plan["loc_gathers"]:
                            if n == 0 or V_SKIP_GATHERS:
                                continue
                            nc.gpsimd.dma_gather(
                                locb[:, cb:cb + n // 128, :], ag_in[:],
                                idx[:, cb * 8: cb * 8 + n // 16],
                                n, n, 128, elem_step=128, single_packet=False,
                                queue_num=next_q())
                        lc = 0
                        while lc < lc_chunks and not V_SKIP_COMPUTE:
                            t = plan["chunk_tile"][lc]
                            ps = locps.tile([128, 128], f32, tag="lps")
                            while lc < lc_chunks and plan["chunk_tile"][lc] == t:
                                sel = selp.tile([128, 128], BF16, tag="sel")
                                nc.vector.tensor_scalar(
                                    sel[:], iota[:], slot[:, lc:lc + 1], val[:, lc:lc + 1],
                                    mybir.AluOpType.is_equal, mybir.AluOpType.mult)
                                nc.tensor.matmul(
                                    ps[:], sel[:], locb[:, lc, :],
                                    start=plan["loc_first"][lc], stop=plan["loc_last"][lc])
                                lc += 1
                            nc.vector.tensor_add(
                                self_buf[:, t * 128:(t + 1) * 128],
                                ps[:], self_buf[:, t * 128:(t + 1) * 128])

                    if not single_core and not V_SKIP_AG:
                        nc.gpsimd.collective_compute(
                            "AllGather", mybir.AluOpType.bypass,
                            replica_groups=[list(range(NCORES))],
                            ins=[ag_in[:].opt()], outs=[ag_out[:].opt()],
                        )
                    # ---- phase 2b: remote gathers + segment sum + combine ----
                    src = xtf_in.ap() if single_core else ag_out[:]
                    src_lo = src[0:split, :]
                    src_hi = src[split:rows_all, :]
                    gi = 0
                    ci = lc_chunks
                    for g in range(tiles // GROUP):
                        (h0, n_lo, cb_lo) = plan["rem_gathers"][gi]
                        (h1, n_hi, cb_hi) = plan["rem_gathers"][gi + 1]
                        gi += 2
                        cg = (n_lo + n_hi) // 128
                        gath = p2.tile([128, cg, 128], BF16, tag="gath")
                        if V_SKIP_GATHERS:
                            nc.vector.memset(gath[:], 0.0)
                        c_lo = n_lo // 128
                        if n_lo and not V_SKIP_GATHERS:
                            nc.gpsimd.dma_gather(
                                gath[:, 0:c_lo, :], src_lo,
                                idx[:, cb_lo * 8: cb_lo * 8 + n_lo // 16],
                                n_lo, n_lo, 128, elem_step=128, single_packet=False,
                                queue_num=next_q())
                        if n_hi and not V_SKIP_GATHERS:
                            nc.gpsimd.dma_gather(
                                gath[:, c_lo:cg, :], src_hi,
                                idx[:, cb_hi * 8: cb_hi * 8 + n_hi // 16],
                                n_hi, n_hi, 128, elem_step=128, single_packet=False,
                                queue_num=next_q())
                        local = list(range(ci, ci + cg))
                        by_tile = {}
                        for lch in local:
                            by_tile.setdefault(plan["chunk_tile"][lch], []).append(lch)
                        for t, lcs in by_tile.items():
                            ps = accps.tile([128, 128], f32, tag="acc")
                            if V_SKIP_COMPUTE:
                                nc.vector.memset(ps[:], 0.0)
                            for lch in lcs:
                                if V_SKIP_COMPUTE:
                                    continue
                                c_in_g = lch - ci
                                ri = lch - lc_chunks
                                sel = selp.tile([128, 128], BF16, tag="sel")
                                nc.vector.tensor_scalar(
                                    sel[:], iota[:], slot[:, lch:lch + 1], val[:, lch:lch + 1],
                                    mybir.AluOpType.is_equal, mybir.AluOpType.mult)
                                nc.tensor.matmul(
                                    ps[:], sel[:], gath[:, c_in_g, :],
                                    start=plan["rem_first"][ri], stop=plan["rem_last"][ri])
                            tmp = outp.tile([128, 128], f32, tag="tmp")
                            nc.vector.tensor_add(
                                tmp[:], ps[:], self_buf[:, t * 128:(t + 1) * 128])
                            ot = outp.tile([128, 128], f32, tag="ot")
                            nc.scalar.activation(
                                ot[:], tmp[:], mybir.ActivationFunctionType.Relu)
                            nc.sync.dma_start(out_d[t * 128:(t + 1) * 128, :], ot[:])
                        ci += cg

            for _rep in range(repeat):
                one_pass()
    nc.compile()
    return nc


def _prepare(x, W, W_self, b_self, edge_row, edge_col, edge_val, repeat=1):
    Bx, N, fin = x.shape
    assert Bx == B and fin == FIN and W.shape == (FIN, FOUT)
    plan = _plan(N, edge_row.astype(np.int64), edge_col.astype(np.int64),
                 edge_val.astype(np.float32))
    npc, npc_pad, tiles = plan["npc"], plan["npc_pad"], plan["tiles"]

    wcat = np.concatenate([W, W_self], axis=1).astype(NP_BF16)
    bias = np.tile(b_self.astype(np.float32)[None, :], (128, 1))
    iota = np.tile(np.arange(128, dtype=np.float32)[None, :], (128, 1)).astype(NP_BF16)

    in_maps = []
    for k in range(NCORES):
        lo = k * npc
        hi = min(N, lo + npc)
        xs = np.zeros((B, npc_pad, FIN), dtype=np.float32)
        xs[:, : hi - lo] = x[:, lo:hi]
        # t-major column layout: col = ((t*B + b)*128 + p)
        xT = xs.reshape(B, tiles, 128, FIN).transpose(3, 1, 0, 2) \
               .reshape(FIN, B * npc_pad).astype(NP_BF16)
        in_maps.append({
            "xT": xT, "wcat": wcat, "bias": bias, "iota": iota,
            "idx": plan["idx_t"][k], "slot": plan["slot_t"][k],
            "val": plan["val_t"][k],
        })

    def assemble(results):
        outs = []
        for k in range(NCORES):
            o = results[k]["out"]
            lo = k * npc
            hi = min(N, lo + npc)
            outs.append(o[: hi - lo].reshape(hi - lo, B, FOUT).transpose(1, 0, 2))
        return np.concatenate(outs, axis=1)

    nc = _build(plan, repeat=repeat)
    return nc, in_maps, assemble


def kernel(x, W, W_self, b_self, edge_row, edge_col, edge_val):
    nc, in_maps, assemble = _prepare(
        np.asarray(x), np.asarray(W), np.asarray(W_self), np.asarray(b_self),
        np.asarray(edge_row), np.asarray(edge_col), np.asarray(edge_val),
    )
    res = run_bass_kernel_spmd(nc, in_maps, core_ids=list(range(NCORES)))
    return assemble(res.results)

